# revision 17
# baseline (speedup 1.0000x reference)
"""GQA attention block (QKV proj + RoPE + attention + out proj) on 8 TRN2 cores.

Sharding: tensor-parallel over heads. Each core gets 4 Q heads + their single
shared KV head (GQA groups intact), plus the matching Wo row-slice. Cores
produce partial [B*S, D] outputs that the host sums.

Per-core dataflow (v3 — fp8-DoubleRow Q proj, fp8 pa, deferred-AV +
quantum-interleaved schedule, engine rebalance):
  - Q proj in fp8e4 DoubleRow (2 k-tiles packed per pass, 0.5 cyc/row;
    halves the Q-proj PE time). DR matmul outputs must start at partition 0
    (ISA: s3d3_mm_valid_dst_partition), so each 64-dim half fills its own
    psum bank and two [64,512] DVE copies (bias fused) assemble qa.
    KV proj stays bf16: V's systematic fp8 W-error would not wash out of
    the final output (Q's does — softmax here is very flat).
  - scores bf16 (as baseline): scoresT[t,s] for a head pair in one 2-bank
    psum tile via row-packed K=64 matmuls; ONE exp [128,1024] per t-tile on
    ACT (the only ACT user -> no act-table swaps), writing pa in fp8e4
    (halves pa SBUF; fp8-moving matmuls run at bf16 speed).
  - AV bf16 ones-trick (as baseline): lhsT = [ones|v.T] so psum rows 0:63
    accumulate the softmax denominator and rows 64:127 o.T. AV for slice ss
    is DEFERRED into slice ss+1's tt loop: head 0's 16 AV matmuls run
    during tts 0-7 (norm emitted at tt 8), head 1's during tts 8-15 — so
    each av bank has ~8 t-tiles of slack before its next-slice reuse.
  - All other PE-only work (projections, O-proj) is queued as ~4k-cycle
    "quanta" drained between score-matmul pairs, filling PE under the exp
    stream and keeping ACT gaps small.
  - ACT runs ONLY exp; all psum->sbuf copies + bias adds + norm on DVE
    (gpsimd cannot access PSUM); SBUF->SBUF kk copy on gpsimd.
  - PSUM (tiles pad to whole 2KB banks): sc 2x[128,1024] (4 banks) +
    av 2x[128,512] (2) + pf 2x[128,512] (2) = 8 banks exactly.
"""

import sys

sys.path.insert(0, "/opt/trn_rl_repo")

from contextlib import ExitStack

import numpy as np
import ml_dtypes

import concourse.bass as bass  # noqa: F401
import concourse.tile as tile
from concourse import bacc, mybir
from concourse.bass_utils import run_bass_kernel_spmd

BF16 = mybir.dt.bfloat16
F32 = mybir.dt.float32
F16 = mybir.dt.float16
FP8 = mybir.dt.float8e4
AF = mybir.ActivationFunctionType
DR = mybir.MatmulPerfMode.DoubleRow

B, S, D = 2, 2048, 2048
QH, KVH, HD = 32, 8, 64
NCORES = 8
QH_LOC = QH // NCORES  # 4 q-heads per core
P = 128
SS = 512  # s-slice (psum free dim)
NSS = S // SS  # 4
KT = D // P  # 16 contraction tiles for projections
NKTP = KT // 2  # 8 double-ktiles for fp8 DR Q proj
NT = S // P  # 16 t-tiles for attention
NPAIR = QH_LOC // 2  # 2 head-pairs per core
SCALE = 1.0 / float(np.sqrt(HD))

# within-head dim permutation: even dims (cos half) first, odd dims second
_PERM = np.concatenate([np.arange(0, HD, 2), np.arange(1, HD, 2)])

DEBUG_DUMPS = False


def _rope(nc, tmp_pool, qsl, cos_sb, sin_sb, head_bases, cols):
    """In-place RoPE on qsl rows [hb, hb+64) for each hb (split-half layout).

    qsl covers sequence columns `cols` (a slice); the tables are indexed with
    the same columns. Both SBUF inputs of each tensor_tensor op must share a
    base partition (walrus verifier); tables are 32-row periodic so any
    aligned row block works.
    """
    width = cols.stop - cols.start
    t1 = tmp_pool.tile([P, width], BF16, tag="ropetmp1")
    t2 = tmp_pool.tile([P, width], BF16, tag="ropetmp2")
    for hb in head_bases:
        lo = slice(hb, hb + 32)
        hi = slice(hb + 32, hb + 64)
        x0 = qsl[lo]
        x1 = qsl[hi]
        nc.vector.tensor_mul(t1[lo], x0, cos_sb[lo, cols])  # x0*cos @ lo
        nc.vector.tensor_mul(t2[lo], x1, sin_sb[hi, cols])  # x1*sin -> lo
        nc.vector.tensor_mul(t1[hi], x0, sin_sb[lo, cols])  # x0*sin -> hi
        nc.vector.tensor_mul(t2[hi], x1, cos_sb[hi, cols])  # x1*cos @ hi
        nc.vector.tensor_sub(x0, t1[lo], t2[lo])
        nc.vector.tensor_add(x1, t1[hi], t2[hi])


def build_nc():
    nc = bacc.Bacc("TRN2", target_bir_lowering=False, debug=False, num_devices=NCORES)

    xt_d = nc.dram_tensor("xt", [B, NSS, P, KT, SS], BF16, kind="ExternalInput")
    wq_d = nc.dram_tensor("wq", [P, KT, NPAIR * P], BF16, kind="ExternalInput")
    wkv_d = nc.dram_tensor("wkv", [P, KT, P], BF16, kind="ExternalInput")
    wo_d = nc.dram_tensor("wo", [P, 2, D], BF16, kind="ExternalInput")
    cos_d = nc.dram_tensor("cost", [P, S], BF16, kind="ExternalInput")
    sin_d = nc.dram_tensor("sint", [P, S], BF16, kind="ExternalInput")
    bq_d = nc.dram_tensor("bq", [P, NPAIR], F32, kind="ExternalInput")
    bkv_d = nc.dram_tensor("bkv", [P, 1], F32, kind="ExternalInput")
    out_d = nc.dram_tensor("out", [B * S, D], F16, kind="ExternalOutput")
    if DEBUG_DUMPS:
        dqa_d = nc.dram_tensor("dqa", [P, B, NPAIR, S], BF16, kind="ExternalOutput")
        dkv_d = nc.dram_tensor("dkv", [P, B, S], BF16, kind="ExternalOutput")
        dva_d = nc.dram_tensor("dva", [P, B, NT, P], BF16, kind="ExternalOutput")
        dot_d = nc.dram_tensor("dot", [P, B, 2, S], BF16, kind="ExternalOutput")

    with tile.TileContext(nc) as tc:
        with ExitStack() as ctx:
            consts = ctx.enter_context(tc.tile_pool(name="consts", bufs=1))
            acts = ctx.enter_context(tc.tile_pool(name="acts", bufs=1))
            xpool = ctx.enter_context(tc.tile_pool(name="xt", bufs=4))
            tmp_pool = ctx.enter_context(tc.tile_pool(name="tmp", bufs=1))
            ppool = ctx.enter_context(tc.tile_pool(name="pexp", bufs=10))
            rpool = ctx.enter_context(tc.tile_pool(name="recip", bufs=1))
            opool = ctx.enter_context(tc.tile_pool(name="osb", bufs=2))
            sc_ps = ctx.enter_context(tc.tile_pool(name="sc", bufs=2, space="PSUM"))
            av_ps = ctx.enter_context(tc.tile_pool(name="av", bufs=2, space="PSUM"))
            pf_ps = ctx.enter_context(tc.tile_pool(name="pf", bufs=2, space="PSUM"))

            # ---- resident constants ----
            wq_sb = consts.tile([P, KT, NPAIR * P], BF16)
            nc.sync.dma_start(wq_sb[:], wq_d.ap())
            wkv_sb = consts.tile([P, KT, P], BF16)
            nc.sync.dma_start(wkv_sb[:], wkv_d.ap())
            wo_sb = consts.tile([P, 2, D], BF16)
            nc.sync.dma_start(wo_sb[:], wo_d.ap())
            cos_sb = consts.tile([P, S], BF16)
            nc.sync.dma_start(cos_sb[:], cos_d.ap())
            sin_sb = consts.tile([P, S], BF16)
            nc.sync.dma_start(sin_sb[:], sin_d.ap())
            bq_sb = consts.tile([P, NPAIR], F32)
            nc.sync.dma_start(bq_sb[:], bq_d.ap())
            bkv_sb = consts.tile([P, 1], F32)
            nc.sync.dma_start(bkv_sb[:], bkv_d.ap())
            # ---- persistent activations ----
            qa_sb = acts.tile([P, B, NPAIR, S], BF16)  # rotated q, pair tiles
            kv_sb = acts.tile([P, B, S], BF16)  # rows 0-63 k(rot), 64-127 v
            kk_sb = acts.tile([P, B, S], BF16)  # rows 64-127 = copy of k
            vaug_sb = acts.tile([P, B, NT, P], BF16)  # [t, 0:64]=1, [64:128]=v
            ot_sb = acts.tile([P, B, 2, S], BF16)  # normalized o.T stacked

            nc.any.memset(vaug_sb[:, :, :, 0:HD], 1.0)

            # ------- PE quantum queue (projection / O-proj work drained
            # between score-matmul pairs to fill PE under the exp stream) ----
            quanta = []

            def drain_quanta(budget):
                while quanta and budget > 0:
                    budget -= quanta.pop(0)()

            # ---------- projection emitters ----------
            def load_xt(b, ss):
                t = xpool.tile([P, KT, SS], BF16, tag="xt")
                nc.sync.dma_start(t[:], xt_d.ap()[b, ss])
                return t

            def kv_fill(b, ss, xt_t):
                """KV proj (bf16) for one s-slice; ~8192c."""
                sl = slice(ss * SS, (ss + 1) * SS)
                ps = pf_ps.tile([P, SS], F32, tag="pf")
                for kt in range(KT):
                    nc.tensor.matmul(
                        ps[:],
                        wkv_sb[:, kt],
                        xt_t[:, kt],
                        start=(kt == 0),
                        stop=(kt == KT - 1),
                    )
                nc.vector.tensor_scalar_add(kv_sb[:, b, sl], ps[:], bkv_sb[:])

            def q_fill(b, pair, ss, xt_t):
                """Q proj (bf16) for one s-slice; ~8192c."""
                sl = slice(ss * SS, (ss + 1) * SS)
                ps = pf_ps.tile([P, SS], F32, tag="pf")
                for kt in range(KT):
                    nc.tensor.matmul(
                        ps[:],
                        wq_sb[:, kt, pair * P : (pair + 1) * P],
                        xt_t[:, kt],
                        start=(kt == 0),
                        stop=(kt == KT - 1),
                    )
                nc.vector.tensor_scalar_add(
                    qa_sb[:, b, pair, sl], ps[:], bq_sb[:, pair : pair + 1]
                )

            def kv_post(b, half):
                """RoPE k + kk copy + v transpose for one half-seq of b."""
                hl = slice(half * 2 * SS, (half + 1) * 2 * SS)
                _rope(nc, tmp_pool, kv_sb[:, b, hl], cos_sb, sin_sb, (0,), hl)
                nc.gpsimd.tensor_copy(kk_sb[HD:P, b, hl], kv_sb[0:HD, b, hl])
                for ci in range(half * 2 * (SS // P), (half + 1) * 2 * (SS // P)):
                    csl = slice(ci * P, (ci + 1) * P)
                    nc.sync.dma_start_transpose(
                        vaug_sb[:, b, ci, HD:P], kv_sb[HD:P, b, csl]
                    )

            def q_rope(b, pair, half):
                hl = slice(half * 2 * SS, (half + 1) * 2 * SS)
                _rope(nc, tmp_pool, qa_sb[:, b, pair, hl], cos_sb, sin_sb, (0, HD), hl)

            def oproj_sc(b, sc_i):
                """O-proj for one 128-row s-chunk: 4 psum fills + gathered
                store; ~4096c."""
                scl = slice(sc_i * P, (sc_i + 1) * P)
                ob = opool.tile([P, D], F16, tag="osb")
                for es in range(NSS):
                    esl = slice(es * SS, (es + 1) * SS)
                    pf = pf_ps.tile([P, SS], F32, tag="pf")
                    for kt2 in range(2):
                        nc.tensor.matmul(
                            pf[:],
                            ot_sb[:, b, kt2, scl],
                            wo_sb[:, kt2, esl],
                            start=(kt2 == 0),
                            stop=(kt2 == 1),
                        )
                    nc.vector.tensor_copy(ob[:, esl], pf[:])
                nc.sync.dma_start(
                    out_d.ap()[b * S + sc_i * P : b * S + (sc_i + 1) * P, :], ob[:]
                )

            # ---------- attention ----------
            # pending = (b, pair, ss, pa_tiles) whose AV+norm still must run
            state = {"pending": None}

            def emit_av(prev, tt, h, av_tiles):
                b, pair, ss, pa_tiles = prev
                nc.tensor.matmul(
                    av_tiles[h][:],
                    vaug_sb[:, b, tt],
                    pa_tiles[tt // 2][:, tt % 2, h * SS : (h + 1) * SS],
                    start=(tt == 0),
                    stop=(tt == NT - 1),
                )

            def emit_norm_head(prev, h, av_tiles):
                b, pair, ss, _ = prev
                t = av_tiles[h]
                cols = slice(ss * SS, (ss + 1) * SS)
                r = rpool.tile([HD, SS], F32, tag="r")
                nc.vector.reciprocal_approx_fast(r[:], t[0:HD])
                nc.vector.tensor_mul(
                    ot_sb[h * HD : (h + 1) * HD, b, pair, cols], t[HD:P], r[:]
                )

            def attn_ss(b, pair, ss, after_norm=None):
                """Scores+exp for (b,pair,ss); AV+norm of the pending slice
                interleaved under the exp stream (head 0's AV chain in tts
                0-7, head 1's in tts 8-15 so av banks get ~8tt of WAR
                slack); quanta drained in the leftover PE slots."""
                prev = state["pending"]
                av_tiles = None
                if prev is not None:
                    av_tiles = [
                        av_ps.tile([P, SS], F32, tag="av", name=f"avt{h}")
                        for h in range(2)
                    ]
                sl = slice(ss * SS, (ss + 1) * SS)
                pa_tiles = []
                for tt in range(NT):
                    csl = slice(tt * P, (tt + 1) * P)
                    sc = sc_ps.tile([P, 2 * SS], F32, tag="sc")
                    nc.tensor.matmul(
                        sc[:, 0:SS],
                        kv_sb[0:HD, b, csl],
                        qa_sb[0:HD, b, pair, sl],
                        start=True,
                        stop=True,
                    )
                    nc.tensor.matmul(
                        sc[:, SS : 2 * SS],
                        kk_sb[HD:P, b, csl],
                        qa_sb[HD:P, b, pair, sl],
                        start=True,
                        stop=True,
                        tile_position=(HD, 0),
                    )
                    if tt % 2 == 0:
                        pa_t = ppool.tile([P, 2, 2 * SS], BF16, tag="pa")
                        pa_tiles.append(pa_t)
                    nc.scalar.activation(
                        pa_tiles[tt // 2][:, tt % 2, :], sc[:], AF.Exp, scale=SCALE
                    )
                    if prev is not None:
                        # delay-2 pacing: AV starts at tt2 so the av banks
                        # have ~3 t-tiles of WAR slack after the previous
                        # slice's norm; tts 2-3 carry double AV load.
                        if tt in (2, 3):
                            ks = (2 * (tt - 2), 2 * (tt - 2) + 1)
                        elif tt >= 4:
                            ks = (tt,)
                        else:
                            ks = ()
                        for k in ks:
                            emit_av(prev, k, 0, av_tiles)
                            emit_av(prev, k, 1, av_tiles)
                        if tt == NT - 1:
                            emit_norm_head(prev, 0, av_tiles)
                            emit_norm_head(prev, 1, av_tiles)
                    if tt % 2 == 1:
                        drain_quanta(2048)
                if prev is not None:
                    cb = state.pop("after_norm", None)
                    if cb is not None:
                        cb()
                state["pending"] = (b, pair, ss, pa_tiles)
                if after_norm is not None:
                    state["after_norm"] = after_norm

            def flush_av():
                prev = state["pending"]
                av_tiles = [
                    av_ps.tile([P, SS], F32, tag="av", name=f"avt{h}")
                    for h in range(2)
                ]
                for h in range(2):
                    for tt in range(NT):
                        emit_av(prev, tt, h, av_tiles)
                    emit_norm_head(prev, h, av_tiles)
                cb = state.pop("after_norm", None)
                if cb is not None:
                    cb()
                state["pending"] = None

            # ---------- emission schedule ----------
            # Prologue: KV(b0) + Q(b0,p0) directly (needed before first
            # scores); everything else queued as quanta. Loads issued early
            # and interleaved so fills don't stall on DMA; b1 tiles are
            # prefetched 2+ quanta ahead of their fills.
            xt_0 = [load_xt(0, 0), load_xt(0, 1), load_xt(0, 2)]
            kv_fill(0, 0, xt_0[0])
            xt_0.append(load_xt(0, 3))
            for ss in range(1, NSS):
                kv_fill(0, ss, xt_0[ss])
                if ss % 2 == 1:
                    kv_post(0, ss // 2)
            for ss in range(NSS):
                q_fill(0, 0, ss, xt_0[ss])
                if ss % 2 == 1:
                    q_rope(0, 0, ss // 2)
            xt_1 = {}

            # quanta: Q(0,p1) -> KV(1) -> Q(1,p0) -> Q(1,p1)
            def q_quantum(b, pair, ss, xt_map, rope_half=None):
                def fn():
                    q_fill(b, pair, ss, xt_map[ss])
                    if rope_half is not None:
                        q_rope(b, pair, rope_half)
                    return 8192

                return fn

            xt_0m = dict(enumerate(xt_0))
            for ss in range(NSS):
                quanta.append(
                    q_quantum(0, 1, ss, xt_0m, ss // 2 if ss % 2 == 1 else None)
                )

            def kv_quantum(b, ss):
                def fn():
                    if ss == 0:
                        xt_1[0] = load_xt(b, 0)
                        xt_1[1] = load_xt(b, 1)
                    if ss + 2 < NSS:
                        xt_1[ss + 2] = load_xt(b, ss + 2)
                    kv_fill(b, ss, xt_1[ss])
                    if ss % 2 == 1:
                        kv_post(b, ss // 2)
                    return 8192

                return fn

            for ss in range(NSS):
                quanta.append(kv_quantum(1, ss))

            for pair in range(2):
                for ss in range(NSS):
                    quanta.append(
                        q_quantum(1, pair, ss, xt_1, ss // 2 if ss % 2 == 1 else None)
                    )

            def oproj_quantum(b, sc_i):
                def fn():
                    oproj_sc(b, sc_i)
                    return 4096

                return fn

            def queue_oproj(b, ss):
                def cb():
                    for sc_i in range(ss * (SS // P), (ss + 1) * (SS // P)):
                        quanta.append(oproj_quantum(b, sc_i))

                return cb

            # attention sweep; O-proj(b, ss) queued once norm(b, pair1, ss)
            # has been emitted (pair order: p0 fully, then p1).
            for ss in range(NSS):
                attn_ss(0, 0, ss)
            for ss in range(NSS):
                attn_ss(0, 1, ss, after_norm=queue_oproj(0, ss))
            for ss in range(NSS):
                attn_ss(1, 0, ss)
            for ss in range(NSS):
                attn_ss(1, 1, ss, after_norm=queue_oproj(1, ss))
            flush_av()
            drain_quanta(1 << 30)

            if DEBUG_DUMPS:
                nc.sync.dma_start(dqa_d.ap(), qa_sb[:])
                nc.sync.dma_start(dkv_d.ap(), kv_sb[:])
                nc.sync.dma_start(dva_d.ap(), vaug_sb[:])
                nc.sync.dma_start(dot_d.ap(), ot_sb[:])

    nc.compile()
    return nc


_NC_CACHE = None


def _get_nc():
    global _NC_CACHE
    if _NC_CACHE is None:
        _NC_CACHE = build_nc()
    return _NC_CACHE


def prepare_in_maps(x, freqs, Wq, bq, Wk, bk, Wv, bv, Wo, bo):
    x = np.asarray(x, np.float32)
    freqs = np.asarray(freqs, np.float32)
    Wq = np.asarray(Wq, np.float32)
    bq = np.asarray(bq, np.float32)
    Wk = np.asarray(Wk, np.float32)
    bk = np.asarray(bk, np.float32)
    Wv = np.asarray(Wv, np.float32)
    bv = np.asarray(bv, np.float32)
    Wo = np.asarray(Wo, np.float32)

    bf = ml_dtypes.bfloat16
    # [B, S, D] -> [B, D, S] -> tiled [B, NSS, P(p), KT(o), SS] with
    # d = o*P + p and s = ss*SS + j, so each (b, ss) DMA is contiguous.
    xt = (
        x.transpose(0, 2, 1)
        .reshape(B, KT, P, NSS, SS)
        .transpose(0, 3, 2, 1, 4)
    )
    xtb = np.ascontiguousarray(xt).astype(bf)
    cost = np.ascontiguousarray(np.tile(freqs[:, :, 0].T, (4, 1))).astype(bf)
    sint = np.ascontiguousarray(np.tile(freqs[:, :, 1].T, (4, 1))).astype(bf)

    in_maps = []
    for c in range(NCORES):
        hq = slice(c * QH_LOC * HD, (c + 1) * QH_LOC * HD)
        hk = slice(c * HD, (c + 1) * HD)
        wq_c = Wq[:, hq].reshape(D, QH_LOC, HD)[:, :, _PERM].reshape(D, QH_LOC * HD)
        bq_c = bq[hq].reshape(QH_LOC, HD)[:, _PERM].reshape(NPAIR, P).T  # [P, NPAIR]
        wk_c = Wk[:, hk][:, _PERM]
        wv_c = Wv[:, hk]
        wkv_c = np.concatenate([wk_c, wv_c], axis=1)
        bkv_c = np.concatenate([bk[hk][_PERM], bv[hk]])[:, None]
        wo_c = Wo[hq, :]
        in_maps.append(
            {
                "xt": xtb,
                "wq": np.ascontiguousarray(
                    wq_c.reshape(KT, P, NPAIR * P).transpose(1, 0, 2)
                ).astype(bf),
                "wkv": np.ascontiguousarray(
                    wkv_c.reshape(KT, P, P).transpose(1, 0, 2)
                ).astype(bf),
                "wo": np.ascontiguousarray(
                    wo_c.reshape(2, P, D).transpose(1, 0, 2)
                ).astype(bf),
                "cost": cost,
                "sint": sint,
                "bq": np.ascontiguousarray(bq_c, dtype=np.float32),
                "bkv": np.ascontiguousarray(bkv_c, dtype=np.float32),
            }
        )
    return in_maps


def run(in_maps, trace=False, **kw):
    nc = _get_nc()
    return run_bass_kernel_spmd(nc, in_maps, list(range(NCORES)), trace=trace, **kw)


def kernel(**inputs):
    in_maps = prepare_in_maps(**{k: inputs[k] for k in (
        "x", "freqs", "Wq", "bq", "Wk", "bk", "Wv", "bv", "Wo", "bo")})
    res = run(in_maps, trace=False)
    acc = np.zeros((B * S, D), np.float64)
    for r in res.results:
        acc += r["out"].astype(np.float64)
    out = acc.astype(np.float32) + np.asarray(inputs["bo"], np.float32)[None, :]
    return out.reshape(B, S, D)
